# revision 2
# baseline (speedup 1.0000x reference)
"""Bidirectional 2-layer ConvLSTM (3x3 grid) + FC head, Trainium2 Bass kernel.

Sharding: data-parallel over batch. B=64 across 8 cores -> 8 batches/core.
Weights replicated; no inter-core communication.

v2 restructure vs baseline:
  - host pre-transposes x to channel-major bf16 (kills PE transposes + PSUM
    round-trip in phase A)
  - recurrence h-conv uses clipped-tap matmuls (49 instead of 81 pixel-MACs)
  - zx folded into the PSUM accumulation via an identity matmul (kills the
    DVE z-add); biases folded into zx during the projection phases
  - gate math per direction in bf16 (DVE 4x mode); hpad copies on GPSIMD
  - zx DRAM scratch in bf16, chunk-loaded 8 steps at a time
  - x-projection phases software-pipelined INTO the recurrence loops
    (program-order interleave keeps PE busy during gate latency)

Per-core phases (single NEFF):
  A) layer-0 input projections (both dirs) -> DRAM zx0   [interleaved: head
     tiles up front, rest inside B's step loop]
  B) layer-0 recurrence, fwd+bwd interleaved per step; h -> SBUF h0store
  C) layer-1 input projections from h0f+h0b -> DRAM zx1  [interleaved into D]
  D) layer-1 recurrence; h -> SBUF h1store
  E) FC head -> out [7, T*BL]
"""

import numpy as np
import ml_dtypes

import concourse.bass as bass
import concourse.mybir as mybir
from concourse.tile import TileContext
from concourse.masks import make_identity

BF16 = mybir.dt.bfloat16
F32 = mybir.dt.float32

B_FULL, T_FULL, C_IN, H, NCLS = 64, 128, 256, 128, 7
NCORES = 8
BL = B_FULL // NCORES  # local batch = 8
CH = 8                 # zx chunk length (timesteps per DMA)

# taps ordered center-first so the first matmul of each accumulation group
# covers every output column (has_written semantics)
TAPS = [(1, 1)] + [(dy, dx) for dy in range(3) for dx in range(3) if (dy, dx) != (1, 1)]
SIG = mybir.ActivationFunctionType.Sigmoid
TANH = mybir.ActivationFunctionType.Tanh
IDENT = mybir.ActivationFunctionType.Identity


def _clip(d):
    # output-pixel range [p0, p0+n) and source range [s0, s0+n) for tap offset d
    if d == 0:
        return 1, 0, 2
    if d == 1:
        return 0, 0, 3
    return 0, 1, 2


def _patch_tile_drain():
    """This walrus rejects >1 sync wait on a Drain: keep the first wait on the
    drain and move the rest onto single-wait NOPs executed just before it."""
    from bass_rust import ScopedClock

    if getattr(TileContext, "_drain_patched", False):
        return

    def _drain_and_barrier(self, tick_clock, wait_clock):
        nc = self.nc
        drain_inst = nc.sync.drain()
        wait_clock.add_sem_waits(
            drain_inst.ins, ScopedClock({None: tick_clock.global_clock})
        )
        si = drain_inst.ins.sync_info
        waits = list(si.on_wait)
        if len(waits) > 1:
            while len(si.on_wait) > 1:
                si.on_wait.pop()
            for w in waits[1:]:
                nop = nc.sync.nop()
                nop.ins.sync_info = mybir.SyncInfo(on_wait=[w], on_update=[])
        nc.all_engine_barrier()
        assert self.sems is not None
        popped = nc._tile_sem_poison_stack.pop()
        assert popped is self._sem_poison
        nc.clear_and_free_semaphores(list(self.sems.allocated().values()))
        nc.all_engine_barrier()

    TileContext._drain_and_barrier = _drain_and_barrier
    TileContext._drain_patched = True


def _fix_multi_waits(raw):
    """This walrus accepts at most 1 sync wait per instruction (2 for
    EventSemaphore). Hoist excess waits onto single-wait EventSemaphore
    carriers inserted just before the instruction on the same engine."""
    import json

    d = json.loads(raw)
    nid = 0
    for fn in d["functions"]:
        for blk in fn["blocks"]:
            out = []
            for inst in blk["instructions"]:
                si = inst.get("sync_info")
                ow = (si or {}).get("on_wait") or []
                cap = 2 if inst.get("opcode") == "EventSemaphore" else 1
                if len(ow) > cap:
                    for w in ow[cap:]:
                        nid += 1
                        out.append({
                            "debug": inst.get("debug", 0),
                            "engine": inst["engine"],
                            "ins": [],
                            "name": f"I-xwait-{nid}",
                            "opcode": "EventSemaphore",
                            "outs": [],
                            "sync_info": {"on_update": [], "on_wait": [w]},
                        })
                    si["on_wait"] = ow[:cap]
                out.append(inst)
            blk["instructions"] = out
    return json.dumps(d).encode()


class _ProjEmitter:
    """Emits one layer's x-projection phase as a sequence of small units so
    the recurrence loop can interleave them between steps.

    Unit granularity: one (gt, dir) block = xpad prep (first dir only) +
    4 sub-tiles x 4 cb_o accumulation groups + the zs->DRAM store.
    Split into `units_per_dirblock` callables.
    """

    # (gt, d) tiles that stay live through the whole recurrence: the far-end
    # tile each direction consumes last
    HEAD_TAGS = {(7, "f"), (0, "b")}

    def __init__(self, nc, layer, blocks, xsrc, wx_sb, bias_sb, zs_tiles, pools, G):
        self.nc = nc
        self.layer = layer      # 0: xsrc is (xcm dram, xpad tiles); 1: (h0store, h0pad tiles)
        self.xsrc = xsrc
        self.wx_sb = wx_sb
        self.bias_sb = bias_sb
        self.zs_tiles = zs_tiles  # shared registry {(gt, d): SBUF tile}
        self.pools = pools      # dict: zp (PSUM), zs ring (SBUF), zs_head (SBUF)
        self.G = G
        self.units = []
        for bi, (gt, d) in enumerate(blocks):
            for sub in range(4):
                self.units.append((bi, gt, d, sub))
        self.pos = 0

    def _prep(self, gt, buf):
        """Stage gt's conv input unpadded: clipped taps never read a border."""
        nc = self.nc
        GT = 128
        ga = gt * GT
        stages = self.xsrc[1]
        xp = stages[buf]
        if self.layer == 0:
            xcm = self.xsrc[0]
            nc.sync.dma_start(out=xp[:], in_=xcm[:, :, ga : ga + GT, :])
        else:
            h0store = self.xsrc[0]
            nc.vector.tensor_add(
                xp[:],
                h0store[:, 0, ga : ga + GT, :],
                h0store[:, 1, ga : ga + GT, :],
            )
        return xp

    def emit_unit(self):
        if self.pos >= len(self.units):
            return False
        nc = self.nc
        bi, gt, d, sub = self.units[self.pos]
        self.pos += 1
        GT = 128
        ga = gt * GT
        xpads = self.xsrc[1]
        buf = bi % 2
        if sub == 0:
            self._prep(gt, buf)
            if (gt, d) in self.HEAD_TAGS:
                self.zs_tiles[(gt, d)] = self.pools["zs_head"].tile(
                    [128, 4, GT, 9], BF16, name=f"zsh{gt}{d}", tag=f"zsh{gt}{d}"
                )
            else:
                self.zs_tiles[(gt, d)] = self.pools["zs"].tile(
                    [128, 4, GT, 9], BF16, name=f"zs{d}", tag=f"zs{d}"
                )
        xp = self.xsrc[1][buf]
        zs = self.zs_tiles[(gt, d)]
        g0 = sub * 32
        for cb_o in range(4):
            zp = self.pools["zp"].tile([128, 32, 3, 3], F32, name="azp", tag="azp")
            zpf = zp[:].rearrange("p g y x -> p (g y x)")
            k = 0
            n_k = 18 if self.layer == 0 else 9
            for dy, dx in TAPS:
                py, sy, ny = _clip(dy)
                px, sx, nx2 = _clip(dx)
                full = ny == 3 and nx2 == 3
                for cb_i in range(2 if self.layer == 0 else 1):
                    if self.layer == 0:
                        w_ap = self.wx_sb[d][
                            :, cb_i, dy * 3 + dx, cb_o * 128 : (cb_o + 1) * 128
                        ]
                        xv = xp[:, cb_i, g0 : g0 + 32, :].rearrange(
                            "p g (y x) -> p g y x", y=3, x=3
                        )
                    else:
                        w_ap = self.wx_sb[d][:, dy * 3 + dx, cb_o * 128 : (cb_o + 1) * 128]
                        xv = xp[:, g0 : g0 + 32, :].rearrange(
                            "p g (y x) -> p g y x", y=3, x=3
                        )
                    r_ap = xv if full else xv[:, :, sy : sy + ny, sx : sx + nx2]
                    o_ap = zpf if full else zp[:, :, py : py + ny, px : px + nx2]
                    nc.tensor.matmul(o_ap, w_ap, r_ap, start=(k == 0), stop=(k == n_k - 1))
                    k += 1
            # PSUM -> SBUF bf16 with bias fold; split ACT/DVE for balance
            dst = zs[:, cb_o, g0 : g0 + 32, :]
            srcv = zp[:].rearrange("p g y x -> p g (y x)")
            bias_ap = self.bias_sb[d][:, cb_o : cb_o + 1]
            if cb_o < 2:
                nc.scalar.activation(dst, srcv, IDENT, bias=bias_ap)
            else:
                nc.vector.tensor_scalar_add(dst, srcv, bias_ap)
        return True


def _recurrence(nc, tc, T, wh_sb, ident_bf, zs_tiles, hstore, pools, emitter, fill_rate, name):
    """One bidirectional ConvLSTM recurrence with interleaved filler units.
    zs_tiles: {(gt, d): SBUF tile [128, 4, 128, 9]} holding x-projections,
    filled by the emitters (head tiles before the loop, the rest JIT)."""
    stp, psp, gp = pools["st"], pools["ps"], pools["g"]
    cst = stp.tile([128, 2, BL * 9], BF16, name=f"{name}c", tag=f"{name}c")
    nc.gpsimd.memset(cst[:], 0.0)

    emitted = 0
    n_units = len(emitter.units) if emitter is not None else 0

    def fill_target(s):
        # deadline-aware spread: ~1 unit/step for the first 8 steps (the
        # first two dir-blocks), then 1 unit per 2 steps
        return min(n_units, (s + 1) if s < 8 else 8 + (s - 6) // 2)

    # PSUM gate-slot order (g, i, f, o): shortens the critical path --
    # tanh(g) is ready first, sigma(i,f) next, sigma(o) only needed for h
    CBS = (3, 0, 1, 2)

    def emit_taps(s, d):
        t = s if d == "f" else T - 1 - s
        tp = t - 1 if d == "f" else t + 1   # previous h timestep
        di = 0 if d == "f" else 1
        zxc = zs_tiles[(t // 16, d)]
        tof = t % 16
        zp = psp.tile([128, 512], F32, name=f"zp{d}", tag=f"{name}zp{d}")
        if s > 0:
            hv = hstore[:, di, tp * BL : (tp + 1) * BL, :].rearrange(
                "p b (y x) -> p b y x", y=3, x=3
            )
        for slot, cb in enumerate(CBS):
            gate = zp[:, slot * 72 : (slot + 1) * 72]
            nc.tensor.matmul(
                gate, ident_bf[:],
                zxc[:, cb, tof * BL : (tof + 1) * BL, :].rearrange("p b yx -> p (b yx)"),
                start=True, stop=(s == 0),
            )
            if s == 0:
                continue   # h(-1) == 0: the conv term vanishes
            g4 = gate.rearrange("p (b y x) -> p b y x", y=3, x=3)
            for kt, (dy, dx) in enumerate(TAPS):
                py, sy, ny = _clip(dy)
                px, sx, nx2 = _clip(dx)
                full = ny == 3 and nx2 == 3
                w_ap = wh_sb[d][:, dy * 3 + dx, cb * 128 : (cb + 1) * 128]
                r_ap = hv if full else hv[:, :, sy : sy + ny, sx : sx + nx2]
                o_ap = gate if full else g4[:, :, py : py + ny, px : px + nx2]
                nc.tensor.matmul(o_ap, w_ap, r_ap, start=False, stop=(kt == 8))
        return zp

    def emit_gates(s, d, zp):
        di = 0 if d == "f" else 1
        t = s if d == "f" else T - 1 - s
        # ACT order: tg, sigma(i,f), sigma(o), then tanh(c) after the c-update
        tg = gp.tile([128, BL * 9], BF16, name=f"tg{d}", tag=f"{name}tg{d}")
        sif = gp.tile([128, 2, BL * 9], BF16, name=f"sif{d}", tag=f"{name}sif{d}")
        so = gp.tile([128, BL * 9], BF16, name=f"so{d}", tag=f"{name}so{d}")
        nc.scalar.activation(tg[:], zp[:, 0:72], TANH)
        nc.scalar.activation(
            sif[:].rearrange("p g b -> p (g b)"), zp[:, 72:216], SIG
        )
        nc.scalar.activation(so[:], zp[:, 216:288], SIG)
        ig = gp.tile([128, BL * 9], BF16, name=f"ig{d}", tag=f"{name}ig{d}")
        cf = gp.tile([128, BL * 9], BF16, name=f"cf{d}", tag=f"{name}cf{d}")
        nc.vector.tensor_mul(ig[:], sif[:, 0], tg[:])
        nc.vector.tensor_mul(cf[:], sif[:, 1], cst[:, di])
        nc.vector.tensor_add(cst[:, di], ig[:], cf[:])
        tct = gp.tile([128, BL * 9], BF16, name=f"tc{d}", tag=f"{name}tc{d}")
        nc.scalar.activation(tct[:], cst[:, di], TANH)
        hsl = hstore[:, di, t * BL : (t + 1) * BL, :].rearrange("p b yx -> p (b yx)")
        nc.vector.tensor_mul(hsl, so[:], tct[:])

    # half-step software pipeline: emit taps(j), then gates(j-1) -- keeps the
    # two directions' chains phase-offset on the shared ACT/DVE engines
    prev = None
    for j in range(2 * T):
        s, d = j // 2, ("f", "b")[j % 2]
        if d == "f" and emitter is not None:
            target = fill_target(s)
            while emitted < target and emitter.emit_unit():
                emitted += 1
        zp = emit_taps(s, d)
        if prev is not None:
            emit_gates(*prev)
        prev = (s, d, zp)
    emit_gates(*prev)
    # drain any unfinished filler units
    if emitter is not None:
        while emitter.emit_unit():
            pass


def build_program(T=T_FULL):
    """Build the per-core Bass program. Returns nc."""
    _patch_tile_drain()
    G = T * BL
    GT = 128
    n_gt = G // GT

    nc = bass.Bass()

    # ---- I/O ----
    xcm = nc.dram_tensor("xcm", [128, 2, G, 9], BF16, kind="ExternalInput")
    wx0 = {}
    wh0 = {}
    wx1 = {}
    wh1 = {}
    bias_in = {}
    for d in ("f", "b"):
        wx0[d] = nc.dram_tensor(f"wx0{d}", [128, 2, 9, 512], BF16, kind="ExternalInput")
        wh0[d] = nc.dram_tensor(f"wh0{d}", [128, 9, 512], BF16, kind="ExternalInput")
        wx1[d] = nc.dram_tensor(f"wx1{d}", [128, 9, 512], BF16, kind="ExternalInput")
        wh1[d] = nc.dram_tensor(f"wh1{d}", [128, 9, 512], BF16, kind="ExternalInput")
        bias_in[f"0{d}"] = nc.dram_tensor(f"bias0{d}", [128, 4], F32, kind="ExternalInput")
        bias_in[f"1{d}"] = nc.dram_tensor(f"bias1{d}", [128, 4], F32, kind="ExternalInput")
    fcw = nc.dram_tensor("fcw", [128, 9, NCLS], BF16, kind="ExternalInput")
    fcb = nc.dram_tensor("fcb", [NCLS, 1], F32, kind="ExternalInput")
    out = nc.dram_tensor("out", [NCLS, G], F32, kind="ExternalOutput")

    with TileContext(nc) as tc:
        with tc.tile_pool(name="persist", bufs=1) as pp:
            bias_sb = {}
            for d in ("f", "b"):
                for l in ("0", "1"):
                    bias_sb[l + d] = pp.tile([128, 4], F32, name=f"bias{l}{d}", tag=f"bias{l}{d}")
                    nc.sync.dma_start(out=bias_sb[l + d][:], in_=bias_in[l + d][:])
            fcb_sb = pp.tile([NCLS, 1], F32, tag="fcb")
            nc.sync.dma_start(out=fcb_sb[:], in_=fcb[:])
            ident_bf = pp.tile([128, 128], BF16, tag="identbf")
            make_identity(nc, ident_bf[:])
            h0store = pp.tile([128, 2, G, 9], BF16, tag="h0store")

            # ===================== layer 0 (A + B) =====================
            with (
                tc.tile_pool(name="l0w", bufs=1) as wp0,
                tc.tile_pool(name="a_zs", bufs=3) as azsp,
                tc.tile_pool(name="a_zsh", bufs=1) as azshp,
                tc.tile_pool(name="a_zp", bufs=3, space="PSUM") as azpp,
                tc.tile_pool(name="b_st", bufs=1) as bstp,
                tc.tile_pool(name="b_ps", bufs=2, space="PSUM") as bpsp,
                tc.tile_pool(name="b_g", bufs=2) as bgp,
            ):
                wx0_sb = {}
                wh0_sb = {}
                for d in ("f", "b"):
                    wx0_sb[d] = wp0.tile([128, 2, 9, 512], BF16, name=f"wx0{d}", tag=f"wx0{d}")
                    nc.sync.dma_start(out=wx0_sb[d][:], in_=wx0[d][:])
                xstage = [
                    wp0.tile([128, 2, GT, 9], BF16, name=f"xst{par}", tag=f"xst{par}")
                    for par in range(2)
                ]

                apools = {"zp": azpp, "zs": azsp, "zs_head": azshp}
                zst0 = {}
                em0 = _ProjEmitter(
                    nc, 0, [(0, "f"), (0, "b"), (7, "f"), (7, "b")], (xcm, xstage), wx0_sb,
                    {d: bias_sb["0" + d] for d in ("f", "b")}, zst0, apools, G,
                )
                while em0.emit_unit():
                    pass
                for d in ("f", "b"):
                    wh0_sb[d] = wp0.tile([128, 9, 512], BF16, name=f"wh0{d}", tag=f"wh0{d}")
                    nc.sync.dma_start(out=wh0_sb[d][:], in_=wh0[d][:])
                em0b = _ProjEmitter(
                    nc, 0,
                    [(1, "f"), (6, "b"), (2, "f"), (5, "b"), (3, "f"), (4, "b"),
                     (4, "f"), (3, "b"), (5, "f"), (2, "b"), (6, "f"), (1, "b")],
                    (xcm, xstage), wx0_sb,
                    {d: bias_sb["0" + d] for d in ("f", "b")}, zst0, apools, G,
                )
                bpools = {"st": bstp, "ps": bpsp, "g": bgp}
                _recurrence(
                    nc, tc, T, wh0_sb, ident_bf, zst0, h0store, bpools,
                    em0b, 2.0, "l0",
                )

            # ===================== layer 1 (C + D + E) =====================
            with tc.tile_pool(name="l1w", bufs=1) as wp1:
                wx1_sb = {}
                wh1_sb = {}
                for d in ("f", "b"):
                    wx1_sb[d] = wp1.tile([128, 9, 512], BF16, name=f"wx1{d}", tag=f"wx1{d}")
                    nc.sync.dma_start(out=wx1_sb[d][:], in_=wx1[d][:])
                    wh1_sb[d] = wp1.tile([128, 9, 512], BF16, name=f"wh1{d}", tag=f"wh1{d}")
                    nc.sync.dma_start(out=wh1_sb[d][:], in_=wh1[d][:])
                fcw_sb = wp1.tile([128, 9, NCLS], BF16, tag="fcw")
                nc.sync.dma_start(out=fcw_sb[:], in_=fcw[:])
                h1store = wp1.tile([128, 2, G, 9], BF16, tag="h1store")
                h0stage = [
                    wp1.tile([128, GT, 9], BF16, name=f"h0st{par}", tag=f"h0st{par}")
                    for par in range(2)
                ]

                with (
                    tc.tile_pool(name="c_zs", bufs=3) as czsp,
                    tc.tile_pool(name="c_zsh", bufs=1) as czshp,
                    tc.tile_pool(name="c_zp", bufs=3, space="PSUM") as czpp,
                    tc.tile_pool(name="d_st", bufs=1) as dstp,
                    tc.tile_pool(name="d_ps", bufs=2, space="PSUM") as dpsp,
                    tc.tile_pool(name="d_g", bufs=2) as dgp,
                ):
                    cpools = {"zp": czpp, "zs": czsp, "zs_head": czshp}
                    zst1 = {}
                    em1 = _ProjEmitter(
                        nc, 1, [(0, "f"), (0, "b"), (7, "f"), (7, "b")], (h0store, h0stage), wx1_sb,
                        {d: bias_sb["1" + d] for d in ("f", "b")}, zst1, cpools, G,
                    )
                    while em1.emit_unit():
                        pass
                    em1b = _ProjEmitter(
                        nc, 1,
                        [(1, "f"), (6, "b"), (2, "f"), (5, "b"), (3, "f"), (4, "b"),
                         (4, "f"), (3, "b"), (5, "f"), (2, "b"), (6, "f"), (1, "b")],
                        (h0store, h0stage), wx1_sb,
                        {d: bias_sb["1" + d] for d in ("f", "b")}, zst1, cpools, G,
                    )
                    dpools = {"st": dstp, "ps": dpsp, "g": dgp}
                    _recurrence(
                        nc, tc, T, wh1_sb, ident_bf, zst1, h1store, dpools,
                        em1b, 2.0, "l1",
                    )

                # ================= Phase E: FC head =================
                with (
                    tc.tile_pool(name="e_h", bufs=2) as ehp,
                    tc.tile_pool(name="e_ps", bufs=1, space="PSUM") as epp,
                    tc.tile_pool(name="e_o", bufs=2) as eop,
                ):
                    EC = 128
                    for gc in range(G // EC):
                        ga = gc * EC
                        h1s = ehp.tile([128, EC, 9], BF16, tag="eh1s")
                        nc.vector.tensor_add(
                            h1s[:],
                            h1store[:, 0, ga : ga + EC, :],
                            h1store[:, 1, ga : ga + EC, :],
                        )
                        ps = epp.tile([NCLS, EC], F32, tag="eps")
                        for yx in range(9):
                            nc.tensor.matmul(
                                ps[:],
                                fcw_sb[:, yx, :],
                                h1s[:, :, yx],
                                start=(yx == 0),
                                stop=(yx == 8),
                            )
                        ot = eop.tile([NCLS, EC], F32, tag="eo")
                        nc.vector.tensor_scalar_add(ot[:], ps[:], fcb_sb[:, 0:1])
                        nc.sync.dma_start(out=out[:, ga : ga + EC], in_=ot[:])

    _orig_to_json = nc.to_json_bytes
    nc.to_json_bytes = lambda: _fix_multi_waits(_orig_to_json())
    return nc


# ---------------- host side ----------------

def _prep_weights(w, b, cin):
    """w: (512, cin+128, 3, 3) -> (wx, wh) bf16 host arrays + bias (128,4) f32."""
    bf = ml_dtypes.bfloat16
    wx = w[:, :cin].reshape(512, cin, 9)            # (co, ci, tap)
    wx = wx.transpose(1, 2, 0)                      # (ci, tap, co)
    if cin == 256:
        wx = wx.reshape(2, 128, 9, 512).transpose(1, 0, 2, 3)  # (128, 2, 9, 512)
    wx = np.ascontiguousarray(wx).astype(bf)
    wh = w[:, cin:].reshape(512, 128, 9).transpose(1, 2, 0)    # (128, 9, 512)
    wh = np.ascontiguousarray(wh).astype(bf)
    bias = np.ascontiguousarray(b.reshape(4, 128).T).astype(np.float32)
    return wx, wh, bias


def make_inputs_core(core, x, w_f0, b_f0, w_b0, b_b0, w_f1, b_f1, w_b1, b_b1,
                     fc_w, fc_b):
    bf = ml_dtypes.bfloat16
    xs = np.ascontiguousarray(x[core * BL : (core + 1) * BL])  # (BL, T, 256, 3, 3)
    T = xs.shape[1]
    xcm = xs.reshape(BL, T, 2, 128, 9).transpose(3, 2, 1, 0, 4)  # (128, 2, T, BL, 9)
    xcm = np.ascontiguousarray(xcm.reshape(128, 2, T * BL, 9)).astype(bf)
    m = {"xcm": xcm}
    for d, w, b in (("f", w_f0, b_f0), ("b", w_b0, b_b0)):
        wx, wh, bias = _prep_weights(np.asarray(w), np.asarray(b), 256)
        m[f"wx0{d}"], m[f"wh0{d}"], m[f"bias0{d}"] = wx, wh, bias
    for d, w, b in (("f", w_f1, b_f1), ("b", w_b1, b_b1)):
        wx, wh, bias = _prep_weights(np.asarray(w), np.asarray(b), 128)
        m[f"wx1{d}"], m[f"wh1{d}"], m[f"bias1{d}"] = wx, wh, bias
    fcw = np.asarray(fc_w).reshape(NCLS, 128, 9).transpose(1, 2, 0)  # (128, 9, 7)
    m["fcw"] = np.ascontiguousarray(fcw).astype(bf)
    m["fcb"] = np.ascontiguousarray(np.asarray(fc_b).reshape(NCLS, 1)).astype(np.float32)
    return m


_nc_cache = {}


def kernel(**inputs):
    from concourse.bass_utils import run_bass_kernel_spmd

    if "nc" not in _nc_cache:
        _nc_cache["nc"] = build_program(T_FULL)
    nc = _nc_cache["nc"]
    x = np.asarray(inputs["x"], dtype=np.float32)
    in_maps = [make_inputs_core(c, x, inputs["w_f0"], inputs["b_f0"],
                                inputs["w_b0"], inputs["b_b0"],
                                inputs["w_f1"], inputs["b_f1"],
                                inputs["w_b1"], inputs["b_b1"],
                                inputs["fc_w"], inputs["fc_b"])
               for c in range(NCORES)]
    res = run_bass_kernel_spmd(nc, in_maps, core_ids=list(range(NCORES)))
    outs = []
    for c in range(NCORES):
        o = res.results[c]["out"]  # (7, G) with g = t*BL + b
        o = o.reshape(NCLS, T_FULL, BL).transpose(2, 1, 0)  # (BL, T, 7)
        outs.append(o)
    return np.ascontiguousarray(np.concatenate(outs, axis=0), dtype=np.float32)


# revision 3
# speedup vs baseline: 1.0382x; 1.0382x over previous
"""Bidirectional 2-layer ConvLSTM (3x3 grid) + FC head, Trainium2 Bass kernel.

Sharding: data-parallel over batch. B=64 across 8 cores -> 8 batches/core.
Weights replicated; no inter-core communication.

v2 restructure vs baseline:
  - host pre-transposes x to channel-major bf16 (kills PE transposes + PSUM
    round-trip in phase A)
  - recurrence h-conv uses clipped-tap matmuls (49 instead of 81 pixel-MACs)
  - zx folded into the PSUM accumulation via an identity matmul (kills the
    DVE z-add); biases folded into zx during the projection phases
  - gate math per direction in bf16 (DVE 4x mode); hpad copies on GPSIMD
  - zx DRAM scratch in bf16, chunk-loaded 8 steps at a time
  - x-projection phases software-pipelined INTO the recurrence loops
    (program-order interleave keeps PE busy during gate latency)

Per-core phases (single NEFF):
  A) layer-0 input projections (both dirs) -> DRAM zx0   [interleaved: head
     tiles up front, rest inside B's step loop]
  B) layer-0 recurrence, fwd+bwd interleaved per step; h -> SBUF h0store
  C) layer-1 input projections from h0f+h0b -> DRAM zx1  [interleaved into D]
  D) layer-1 recurrence; h -> SBUF h1store
  E) FC head -> out [7, T*BL]
"""

import numpy as np
import ml_dtypes

import concourse.bass as bass
import concourse.mybir as mybir
from concourse.tile import TileContext
from concourse.masks import make_identity

BF16 = mybir.dt.bfloat16
F32 = mybir.dt.float32

B_FULL, T_FULL, C_IN, H, NCLS = 64, 128, 256, 128, 7
NCORES = 8
BL = B_FULL // NCORES  # local batch = 8
CH = 8                 # zx chunk length (timesteps per DMA)

# taps ordered center-first so the first matmul of each accumulation group
# covers every output column (has_written semantics)
TAPS = [(1, 1)] + [(dy, dx) for dy in range(3) for dx in range(3) if (dy, dx) != (1, 1)]
SIG = mybir.ActivationFunctionType.Sigmoid
TANH = mybir.ActivationFunctionType.Tanh
IDENT = mybir.ActivationFunctionType.Identity


def _clip(d):
    # output-pixel range [p0, p0+n) and source range [s0, s0+n) for tap offset d
    if d == 0:
        return 1, 0, 2
    if d == 1:
        return 0, 0, 3
    return 0, 1, 2


def _patch_tile_drain():
    """This walrus rejects >1 sync wait on a Drain: keep the first wait on the
    drain and move the rest onto single-wait NOPs executed just before it."""
    from bass_rust import ScopedClock

    if getattr(TileContext, "_drain_patched", False):
        return

    def _drain_and_barrier(self, tick_clock, wait_clock):
        nc = self.nc
        drain_inst = nc.sync.drain()
        wait_clock.add_sem_waits(
            drain_inst.ins, ScopedClock({None: tick_clock.global_clock})
        )
        si = drain_inst.ins.sync_info
        waits = list(si.on_wait)
        if len(waits) > 1:
            while len(si.on_wait) > 1:
                si.on_wait.pop()
            for w in waits[1:]:
                nop = nc.sync.nop()
                nop.ins.sync_info = mybir.SyncInfo(on_wait=[w], on_update=[])
        nc.all_engine_barrier()
        assert self.sems is not None
        popped = nc._tile_sem_poison_stack.pop()
        assert popped is self._sem_poison
        nc.clear_and_free_semaphores(list(self.sems.allocated().values()))
        nc.all_engine_barrier()

    TileContext._drain_and_barrier = _drain_and_barrier
    TileContext._drain_patched = True


def _fix_multi_waits(raw):
    """This walrus accepts at most 1 sync wait per instruction (2 for
    EventSemaphore). Hoist excess waits onto single-wait EventSemaphore
    carriers inserted just before the instruction on the same engine."""
    import json

    d = json.loads(raw)
    nid = 0
    for fn in d["functions"]:
        for blk in fn["blocks"]:
            out = []
            for inst in blk["instructions"]:
                si = inst.get("sync_info")
                ow = (si or {}).get("on_wait") or []
                cap = 2 if inst.get("opcode") == "EventSemaphore" else 1
                if len(ow) > cap:
                    for w in ow[cap:]:
                        nid += 1
                        out.append({
                            "debug": inst.get("debug", 0),
                            "engine": inst["engine"],
                            "ins": [],
                            "name": f"I-xwait-{nid}",
                            "opcode": "EventSemaphore",
                            "outs": [],
                            "sync_info": {"on_update": [], "on_wait": [w]},
                        })
                    si["on_wait"] = ow[:cap]
                out.append(inst)
            blk["instructions"] = out
    return json.dumps(d).encode()


class _ProjEmitter:
    """Emits one layer's x-projection phase as a sequence of small units so
    the recurrence loop can interleave them between steps.

    Unit granularity: one (gt, dir) block = xpad prep (first dir only) +
    4 sub-tiles x 4 cb_o accumulation groups + the zs->DRAM store.
    Split into `units_per_dirblock` callables.
    """

    # (gt, d) tiles that stay live through the whole recurrence: the far-end
    # tile each direction consumes last
    HEAD_TAGS = {(7, "f"), (0, "b")}

    def __init__(self, nc, layer, blocks, xsrc, wx_sb, bias_sb, zs_tiles, pools, G):
        self.nc = nc
        self.layer = layer      # 0: xsrc is (xcm dram, xpad tiles); 1: (h0store, h0pad tiles)
        self.xsrc = xsrc
        self.wx_sb = wx_sb
        self.bias_sb = bias_sb
        self.zs_tiles = zs_tiles  # shared registry {(gt, d): SBUF tile}
        self.pools = pools      # dict: zp (PSUM), zs ring (SBUF), zs_head (SBUF)
        self.G = G
        self.units = []
        for bi, (gt, d) in enumerate(blocks):
            for sub in range(4):
                self.units.append((bi, gt, d, sub))
        self.pos = 0

    def _prep(self, gt, buf):
        """Stage gt's conv input unpadded: clipped taps never read a border."""
        nc = self.nc
        GT = 128
        ga = gt * GT
        stages = self.xsrc[1]
        xp = stages[buf]
        if self.layer == 0:
            xcm = self.xsrc[0]
            nc.sync.dma_start(out=xp[:], in_=xcm[:, :, ga : ga + GT, :])
        else:
            h0store = self.xsrc[0]
            nc.vector.tensor_add(
                xp[:],
                h0store[:, 0, ga : ga + GT, :],
                h0store[:, 1, ga : ga + GT, :],
            )
        return xp

    def emit_unit(self):
        if self.pos >= len(self.units):
            return False
        nc = self.nc
        bi, gt, d, sub = self.units[self.pos]
        self.pos += 1
        GT = 128
        ga = gt * GT
        xpads = self.xsrc[1]
        buf = bi % 2
        if sub == 0:
            self._prep(gt, buf)
            if (gt, d) in self.HEAD_TAGS:
                self.zs_tiles[(gt, d)] = self.pools["zs_head"].tile(
                    [128, 4, GT, 9], BF16, name=f"zsh{gt}{d}", tag=f"zsh{gt}{d}"
                )
            else:
                self.zs_tiles[(gt, d)] = self.pools["zs"].tile(
                    [128, 4, GT, 9], BF16, name=f"zs{d}", tag=f"zs{d}"
                )
        xp = self.xsrc[1][buf]
        zs = self.zs_tiles[(gt, d)]
        g0 = sub * 32
        for cb_o in range(4):
            zp = self.pools["zp"].tile([128, 32, 3, 3], F32, name="azp", tag="azp")
            zpf = zp[:].rearrange("p g y x -> p (g y x)")
            k = 0
            n_k = 18 if self.layer == 0 else 9
            for dy, dx in TAPS:
                py, sy, ny = _clip(dy)
                px, sx, nx2 = _clip(dx)
                full = ny == 3 and nx2 == 3
                for cb_i in range(2 if self.layer == 0 else 1):
                    if self.layer == 0:
                        w_ap = self.wx_sb[d][
                            :, cb_i, dy * 3 + dx, cb_o * 128 : (cb_o + 1) * 128
                        ]
                        xv = xp[:, cb_i, g0 : g0 + 32, :].rearrange(
                            "p g (y x) -> p g y x", y=3, x=3
                        )
                    else:
                        w_ap = self.wx_sb[d][:, dy * 3 + dx, cb_o * 128 : (cb_o + 1) * 128]
                        xv = xp[:, g0 : g0 + 32, :].rearrange(
                            "p g (y x) -> p g y x", y=3, x=3
                        )
                    r_ap = xv if full else xv[:, :, sy : sy + ny, sx : sx + nx2]
                    o_ap = zpf if full else zp[:, :, py : py + ny, px : px + nx2]
                    nc.tensor.matmul(o_ap, w_ap, r_ap, start=(k == 0), stop=(k == n_k - 1))
                    k += 1
            # PSUM -> SBUF bf16 with bias fold; split ACT/DVE for balance
            dst = zs[:, cb_o, g0 : g0 + 32, :]
            srcv = zp[:].rearrange("p g y x -> p g (y x)")
            bias_ap = self.bias_sb[d][:, cb_o : cb_o + 1]
            if cb_o < 2:
                nc.scalar.activation(dst, srcv, IDENT, bias=bias_ap)
            else:
                nc.vector.tensor_scalar_add(dst, srcv, bias_ap)
        return True


def _recurrence(nc, tc, T, wh_sb, ident_bf, zs_tiles, hstore, pools, emitter, fill_rate, name):
    """One bidirectional ConvLSTM recurrence with interleaved filler units.
    zs_tiles: {(gt, d): SBUF tile [128, 4, 128, 9]} holding x-projections,
    filled by the emitters (head tiles before the loop, the rest JIT)."""
    stp, psp, gp = pools["st"], pools["ps"], pools["g"]
    cst = stp.tile([128, 2, BL * 9], BF16, name=f"{name}c", tag=f"{name}c")
    nc.gpsimd.memset(cst[:], 0.0)

    emitted = 0
    n_units = len(emitter.units) if emitter is not None else 0

    def fill_target(s):
        # deadline-aware spread: ~1 unit/step for the first 8 steps (the
        # first two dir-blocks), then 1 unit per 2 steps
        return min(n_units, (s + 1) if s < 8 else 8 + (s - 6) // 2)

    # PSUM gate-slot order (g, i, f, o): shortens the critical path --
    # tanh(g) is ready first, sigma(i,f) next, sigma(o) only needed for h
    CBS = (3, 0, 1, 2)

    def emit_taps(s, d):
        t = s if d == "f" else T - 1 - s
        tp = t - 1 if d == "f" else t + 1   # previous h timestep
        di = 0 if d == "f" else 1
        zxc = zs_tiles[(t // 16, d)]
        tof = t % 16
        zp = psp.tile([128, 512], F32, name=f"zp{d}", tag=f"{name}zp{d}")
        if s > 0:
            hv = hstore[:, di, tp * BL : (tp + 1) * BL, :].rearrange(
                "p b (y x) -> p b y x", y=3, x=3
            )
        for slot, cb in enumerate(CBS):
            gate = zp[:, slot * 72 : (slot + 1) * 72]
            nc.tensor.matmul(
                gate, ident_bf[:],
                zxc[:, cb, tof * BL : (tof + 1) * BL, :].rearrange("p b yx -> p (b yx)"),
                start=True, stop=(s == 0),
            )
            if s == 0:
                continue   # h(-1) == 0: the conv term vanishes
            g4 = gate.rearrange("p (b y x) -> p b y x", y=3, x=3)
            for kt, (dy, dx) in enumerate(TAPS):
                py, sy, ny = _clip(dy)
                px, sx, nx2 = _clip(dx)
                full = ny == 3 and nx2 == 3
                w_ap = wh_sb[d][:, dy * 3 + dx, cb * 128 : (cb + 1) * 128]
                r_ap = hv if full else hv[:, :, sy : sy + ny, sx : sx + nx2]
                o_ap = gate if full else g4[:, :, py : py + ny, px : px + nx2]
                nc.tensor.matmul(o_ap, w_ap, r_ap, start=False, stop=(kt == 8))
        return zp

    def emit_gates(s, d, zp):
        di = 0 if d == "f" else 1
        t = s if d == "f" else T - 1 - s
        # ACT order: tg, sigma(i,f), sigma(o), then tanh(c) after the c-update
        tg = gp.tile([128, BL * 9], BF16, name=f"tg{d}", tag=f"{name}tg{d}")
        sif = gp.tile([128, 2, BL * 9], BF16, name=f"sif{d}", tag=f"{name}sif{d}")
        so = gp.tile([128, BL * 9], BF16, name=f"so{d}", tag=f"{name}so{d}")
        nc.scalar.activation(tg[:], zp[:, 0:72], TANH)
        nc.scalar.activation(
            sif[:].rearrange("p g b -> p (g b)"), zp[:, 72:216], SIG
        )
        nc.scalar.activation(so[:], zp[:, 216:288], SIG)
        ig = gp.tile([128, BL * 9], BF16, name=f"ig{d}", tag=f"{name}ig{d}")
        cf = gp.tile([128, BL * 9], BF16, name=f"cf{d}", tag=f"{name}cf{d}")
        nc.vector.tensor_mul(ig[:], sif[:, 0], tg[:])
        nc.vector.tensor_mul(cf[:], sif[:, 1], cst[:, di])
        nc.vector.tensor_add(cst[:, di], ig[:], cf[:])
        tct = gp.tile([128, BL * 9], BF16, name=f"tc{d}", tag=f"{name}tc{d}")
        nc.scalar.activation(tct[:], cst[:, di], TANH)
        hsl = hstore[:, di, t * BL : (t + 1) * BL, :].rearrange("p b yx -> p (b yx)")
        nc.vector.tensor_mul(hsl, so[:], tct[:])

    # half-step software pipeline: emit taps(j), then gates(j-1) -- keeps the
    # two directions' chains phase-offset on the shared ACT/DVE engines
    prev = None
    for j in range(2 * T):
        s, d = j // 2, ("f", "b")[j % 2]
        if d == "f" and emitter is not None:
            target = fill_target(s)
            while emitted < target and emitter.emit_unit():
                emitted += 1
        zp = emit_taps(s, d)
        if prev is not None:
            emit_gates(*prev)
        prev = (s, d, zp)
    emit_gates(*prev)
    # drain any unfinished filler units
    if emitter is not None:
        while emitter.emit_unit():
            pass


def build_program(T=T_FULL):
    """Build the per-core Bass program. Returns nc."""
    _patch_tile_drain()
    G = T * BL
    GT = 128
    n_gt = G // GT

    nc = bass.Bass()

    # ---- I/O ----
    xcm = nc.dram_tensor("xcm", [128, 2, G, 9], BF16, kind="ExternalInput")
    wx0 = {}
    wh0 = {}
    wx1 = {}
    wh1 = {}
    bias_in = {}
    for d in ("f", "b"):
        wx0[d] = nc.dram_tensor(f"wx0{d}", [128, 2, 9, 512], BF16, kind="ExternalInput")
        wh0[d] = nc.dram_tensor(f"wh0{d}", [128, 9, 512], BF16, kind="ExternalInput")
        wx1[d] = nc.dram_tensor(f"wx1{d}", [128, 9, 512], BF16, kind="ExternalInput")
        wh1[d] = nc.dram_tensor(f"wh1{d}", [128, 9, 512], BF16, kind="ExternalInput")
        bias_in[f"0{d}"] = nc.dram_tensor(f"bias0{d}", [128, 4], F32, kind="ExternalInput")
        bias_in[f"1{d}"] = nc.dram_tensor(f"bias1{d}", [128, 4], F32, kind="ExternalInput")
    fcw = nc.dram_tensor("fcw", [128, 9, NCLS], BF16, kind="ExternalInput")
    fcb = nc.dram_tensor("fcb", [NCLS, 1], F32, kind="ExternalInput")
    out = nc.dram_tensor("out", [NCLS, G], F32, kind="ExternalOutput")

    with TileContext(nc) as tc:
        with tc.tile_pool(name="persist", bufs=1) as pp:
            bias_sb = {}
            for d in ("f", "b"):
                for l in ("0", "1"):
                    bias_sb[l + d] = pp.tile([128, 4], F32, name=f"bias{l}{d}", tag=f"bias{l}{d}")
                    nc.sync.dma_start(out=bias_sb[l + d][:], in_=bias_in[l + d][:])
            fcb_sb = pp.tile([NCLS, 1], F32, tag="fcb")
            nc.sync.dma_start(out=fcb_sb[:], in_=fcb[:])
            ident_bf = pp.tile([128, 128], BF16, tag="identbf")
            make_identity(nc, ident_bf[:])
            h0store = pp.tile([128, 2, G, 9], BF16, tag="h0store")

            # ===================== layer 0 (A + B) =====================
            with (
                tc.tile_pool(name="l0w", bufs=1) as wp0,
                tc.tile_pool(name="a_zs", bufs=3) as azsp,
                tc.tile_pool(name="a_zsh", bufs=1) as azshp,
                tc.tile_pool(name="a_zp", bufs=3, space="PSUM") as azpp,
                tc.tile_pool(name="b_st", bufs=1) as bstp,
                tc.tile_pool(name="b_ps", bufs=2, space="PSUM") as bpsp,
                tc.tile_pool(name="b_g", bufs=2) as bgp,
            ):
                wx0_sb = {}
                wh0_sb = {}
                for d in ("f", "b"):
                    wx0_sb[d] = wp0.tile([128, 2, 9, 512], BF16, name=f"wx0{d}", tag=f"wx0{d}")
                    nc.sync.dma_start(out=wx0_sb[d][:], in_=wx0[d][:])
                xstage = [
                    wp0.tile([128, 2, GT, 9], BF16, name=f"xst{par}", tag=f"xst{par}")
                    for par in range(2)
                ]

                apools = {"zp": azpp, "zs": azsp, "zs_head": azshp}
                zst0 = {}
                em0 = _ProjEmitter(
                    nc, 0, [(0, "f"), (7, "b")], (xcm, xstage), wx0_sb,
                    {d: bias_sb["0" + d] for d in ("f", "b")}, zst0, apools, G,
                )
                while em0.emit_unit():
                    pass
                for d in ("f", "b"):
                    wh0_sb[d] = wp0.tile([128, 9, 512], BF16, name=f"wh0{d}", tag=f"wh0{d}")
                    nc.sync.dma_start(out=wh0_sb[d][:], in_=wh0[d][:])
                em0b = _ProjEmitter(
                    nc, 0,
                    [(1, "f"), (6, "b"), (2, "f"), (5, "b"), (3, "f"), (4, "b"),
                     (4, "f"), (3, "b"), (5, "f"), (2, "b"), (6, "f"), (1, "b"),
                     (7, "f"), (0, "b")],
                    (xcm, xstage), wx0_sb,
                    {d: bias_sb["0" + d] for d in ("f", "b")}, zst0, apools, G,
                )
                bpools = {"st": bstp, "ps": bpsp, "g": bgp}
                _recurrence(
                    nc, tc, T, wh0_sb, ident_bf, zst0, h0store, bpools,
                    em0b, 2.0, "l0",
                )

            # ===================== layer 1 (C + D + E) =====================
            with tc.tile_pool(name="l1w", bufs=1) as wp1:
                wx1_sb = {}
                wh1_sb = {}
                for d in ("f", "b"):
                    wx1_sb[d] = wp1.tile([128, 9, 512], BF16, name=f"wx1{d}", tag=f"wx1{d}")
                    nc.sync.dma_start(out=wx1_sb[d][:], in_=wx1[d][:])
                    wh1_sb[d] = wp1.tile([128, 9, 512], BF16, name=f"wh1{d}", tag=f"wh1{d}")
                    nc.sync.dma_start(out=wh1_sb[d][:], in_=wh1[d][:])
                fcw_sb = wp1.tile([128, 9, NCLS], BF16, tag="fcw")
                nc.sync.dma_start(out=fcw_sb[:], in_=fcw[:])
                h1store = wp1.tile([128, 2, G, 9], BF16, tag="h1store")
                h0stage = [
                    wp1.tile([128, GT, 9], BF16, name=f"h0st{par}", tag=f"h0st{par}")
                    for par in range(2)
                ]

                with (
                    tc.tile_pool(name="c_zs", bufs=3) as czsp,
                    tc.tile_pool(name="c_zsh", bufs=1) as czshp,
                    tc.tile_pool(name="c_zp", bufs=3, space="PSUM") as czpp,
                    tc.tile_pool(name="d_st", bufs=1) as dstp,
                    tc.tile_pool(name="d_ps", bufs=2, space="PSUM") as dpsp,
                    tc.tile_pool(name="d_g", bufs=2) as dgp,
                ):
                    cpools = {"zp": czpp, "zs": czsp, "zs_head": czshp}
                    zst1 = {}
                    em1 = _ProjEmitter(
                        nc, 1, [(0, "f"), (7, "b")], (h0store, h0stage), wx1_sb,
                        {d: bias_sb["1" + d] for d in ("f", "b")}, zst1, cpools, G,
                    )
                    while em1.emit_unit():
                        pass
                    em1b = _ProjEmitter(
                        nc, 1,
                        [(1, "f"), (6, "b"), (2, "f"), (5, "b"), (3, "f"), (4, "b"),
                         (4, "f"), (3, "b"), (5, "f"), (2, "b"), (6, "f"), (1, "b"),
                         (7, "f"), (0, "b")],
                        (h0store, h0stage), wx1_sb,
                        {d: bias_sb["1" + d] for d in ("f", "b")}, zst1, cpools, G,
                    )
                    dpools = {"st": dstp, "ps": dpsp, "g": dgp}
                    _recurrence(
                        nc, tc, T, wh1_sb, ident_bf, zst1, h1store, dpools,
                        em1b, 2.0, "l1",
                    )

                # ================= Phase E: FC head =================
                with (
                    tc.tile_pool(name="e_h", bufs=2) as ehp,
                    tc.tile_pool(name="e_ps", bufs=1, space="PSUM") as epp,
                    tc.tile_pool(name="e_o", bufs=2) as eop,
                ):
                    EC = 128
                    for gc in (3, 4, 2, 5, 1, 6, 0, 7):
                        ga = gc * EC
                        h1s = ehp.tile([128, EC, 9], BF16, tag="eh1s")
                        nc.vector.tensor_add(
                            h1s[:],
                            h1store[:, 0, ga : ga + EC, :],
                            h1store[:, 1, ga : ga + EC, :],
                        )
                        ps = epp.tile([NCLS, EC], F32, tag="eps")
                        for yx in range(9):
                            nc.tensor.matmul(
                                ps[:],
                                fcw_sb[:, yx, :],
                                h1s[:, :, yx],
                                start=(yx == 0),
                                stop=(yx == 8),
                            )
                        ot = eop.tile([NCLS, EC], F32, tag="eo")
                        nc.vector.tensor_scalar_add(ot[:], ps[:], fcb_sb[:, 0:1])
                        nc.sync.dma_start(out=out[:, ga : ga + EC], in_=ot[:])

    _orig_to_json = nc.to_json_bytes
    nc.to_json_bytes = lambda: _fix_multi_waits(_orig_to_json())
    return nc


# ---------------- host side ----------------

def _prep_weights(w, b, cin):
    """w: (512, cin+128, 3, 3) -> (wx, wh) bf16 host arrays + bias (128,4) f32."""
    bf = ml_dtypes.bfloat16
    wx = w[:, :cin].reshape(512, cin, 9)            # (co, ci, tap)
    wx = wx.transpose(1, 2, 0)                      # (ci, tap, co)
    if cin == 256:
        wx = wx.reshape(2, 128, 9, 512).transpose(1, 0, 2, 3)  # (128, 2, 9, 512)
    wx = np.ascontiguousarray(wx).astype(bf)
    wh = w[:, cin:].reshape(512, 128, 9).transpose(1, 2, 0)    # (128, 9, 512)
    wh = np.ascontiguousarray(wh).astype(bf)
    bias = np.ascontiguousarray(b.reshape(4, 128).T).astype(np.float32)
    return wx, wh, bias


def make_inputs_core(core, x, w_f0, b_f0, w_b0, b_b0, w_f1, b_f1, w_b1, b_b1,
                     fc_w, fc_b):
    bf = ml_dtypes.bfloat16
    xs = np.ascontiguousarray(x[core * BL : (core + 1) * BL])  # (BL, T, 256, 3, 3)
    T = xs.shape[1]
    xcm = xs.reshape(BL, T, 2, 128, 9).transpose(3, 2, 1, 0, 4)  # (128, 2, T, BL, 9)
    xcm = np.ascontiguousarray(xcm.reshape(128, 2, T * BL, 9)).astype(bf)
    m = {"xcm": xcm}
    for d, w, b in (("f", w_f0, b_f0), ("b", w_b0, b_b0)):
        wx, wh, bias = _prep_weights(np.asarray(w), np.asarray(b), 256)
        m[f"wx0{d}"], m[f"wh0{d}"], m[f"bias0{d}"] = wx, wh, bias
    for d, w, b in (("f", w_f1, b_f1), ("b", w_b1, b_b1)):
        wx, wh, bias = _prep_weights(np.asarray(w), np.asarray(b), 128)
        m[f"wx1{d}"], m[f"wh1{d}"], m[f"bias1{d}"] = wx, wh, bias
    fcw = np.asarray(fc_w).reshape(NCLS, 128, 9).transpose(1, 2, 0)  # (128, 9, 7)
    m["fcw"] = np.ascontiguousarray(fcw).astype(bf)
    m["fcb"] = np.ascontiguousarray(np.asarray(fc_b).reshape(NCLS, 1)).astype(np.float32)
    return m


_nc_cache = {}


def kernel(**inputs):
    from concourse.bass_utils import run_bass_kernel_spmd

    if "nc" not in _nc_cache:
        _nc_cache["nc"] = build_program(T_FULL)
    nc = _nc_cache["nc"]
    x = np.asarray(inputs["x"], dtype=np.float32)
    in_maps = [make_inputs_core(c, x, inputs["w_f0"], inputs["b_f0"],
                                inputs["w_b0"], inputs["b_b0"],
                                inputs["w_f1"], inputs["b_f1"],
                                inputs["w_b1"], inputs["b_b1"],
                                inputs["fc_w"], inputs["fc_b"])
               for c in range(NCORES)]
    res = run_bass_kernel_spmd(nc, in_maps, core_ids=list(range(NCORES)))
    outs = []
    for c in range(NCORES):
        o = res.results[c]["out"]  # (7, G) with g = t*BL + b
        o = o.reshape(NCLS, T_FULL, BL).transpose(2, 1, 0)  # (BL, T, 7)
        outs.append(o)
    return np.ascontiguousarray(np.concatenate(outs, axis=0), dtype=np.float32)


# revision 5
# speedup vs baseline: 1.0745x; 1.0349x over previous
"""Bidirectional 2-layer ConvLSTM (3x3 grid) + FC head, Trainium2 Bass kernel.

Sharding: data-parallel over batch. B=64 across 8 cores -> 8 batches/core.
Weights replicated; no inter-core communication.

v2 restructure vs baseline:
  - host pre-transposes x to channel-major bf16 (kills PE transposes + PSUM
    round-trip in phase A)
  - recurrence h-conv uses clipped-tap matmuls (49 instead of 81 pixel-MACs)
  - zx folded into the PSUM accumulation via an identity matmul (kills the
    DVE z-add); biases folded into zx during the projection phases
  - gate math per direction in bf16 (DVE 4x mode); hpad copies on GPSIMD
  - zx DRAM scratch in bf16, chunk-loaded 8 steps at a time
  - x-projection phases software-pipelined INTO the recurrence loops
    (program-order interleave keeps PE busy during gate latency)

Per-core phases (single NEFF):
  A) layer-0 input projections (both dirs) -> DRAM zx0   [interleaved: head
     tiles up front, rest inside B's step loop]
  B) layer-0 recurrence, fwd+bwd interleaved per step; h -> SBUF h0store
  C) layer-1 input projections from h0f+h0b -> DRAM zx1  [interleaved into D]
  D) layer-1 recurrence; h -> SBUF h1store
  E) FC head -> out [7, T*BL]
"""

import numpy as np
import ml_dtypes

import concourse.bass as bass
import concourse.mybir as mybir
from concourse.tile import TileContext
from concourse.masks import make_identity

BF16 = mybir.dt.bfloat16
F32 = mybir.dt.float32

B_FULL, T_FULL, C_IN, H, NCLS = 64, 128, 256, 128, 7
NCORES = 8
BL = B_FULL // NCORES  # local batch = 8
CH = 8                 # zx chunk length (timesteps per DMA)

# taps ordered center-first so the first matmul of each accumulation group
# covers every output column (has_written semantics)
TAPS = [(1, 1)] + [(dy, dx) for dy in range(3) for dx in range(3) if (dy, dx) != (1, 1)]
SIG = mybir.ActivationFunctionType.Sigmoid
TANH = mybir.ActivationFunctionType.Tanh
IDENT = mybir.ActivationFunctionType.Identity


def _clip(d):
    # output-pixel range [p0, p0+n) and source range [s0, s0+n) for tap offset d
    if d == 0:
        return 1, 0, 2
    if d == 1:
        return 0, 0, 3
    return 0, 1, 2


def _patch_tile_drain():
    """This walrus rejects >1 sync wait on a Drain: keep the first wait on the
    drain and move the rest onto single-wait NOPs executed just before it."""
    from bass_rust import ScopedClock

    if getattr(TileContext, "_drain_patched", False):
        return

    def _drain_and_barrier(self, tick_clock, wait_clock):
        nc = self.nc
        drain_inst = nc.sync.drain()
        wait_clock.add_sem_waits(
            drain_inst.ins, ScopedClock({None: tick_clock.global_clock})
        )
        si = drain_inst.ins.sync_info
        waits = list(si.on_wait)
        if len(waits) > 1:
            while len(si.on_wait) > 1:
                si.on_wait.pop()
            for w in waits[1:]:
                nop = nc.sync.nop()
                nop.ins.sync_info = mybir.SyncInfo(on_wait=[w], on_update=[])
        nc.all_engine_barrier()
        assert self.sems is not None
        popped = nc._tile_sem_poison_stack.pop()
        assert popped is self._sem_poison
        nc.clear_and_free_semaphores(list(self.sems.allocated().values()))
        nc.all_engine_barrier()

    TileContext._drain_and_barrier = _drain_and_barrier
    TileContext._drain_patched = True


def _fix_multi_waits(raw):
    """This walrus accepts at most 1 sync wait per instruction (2 for
    EventSemaphore). Hoist excess waits onto single-wait EventSemaphore
    carriers inserted just before the instruction on the same engine."""
    import json

    d = json.loads(raw)
    nid = 0
    for fn in d["functions"]:
        for blk in fn["blocks"]:
            out = []
            for inst in blk["instructions"]:
                si = inst.get("sync_info")
                ow = (si or {}).get("on_wait") or []
                cap = 2 if inst.get("opcode") == "EventSemaphore" else 1
                if len(ow) > cap:
                    for w in ow[cap:]:
                        nid += 1
                        out.append({
                            "debug": inst.get("debug", 0),
                            "engine": inst["engine"],
                            "ins": [],
                            "name": f"I-xwait-{nid}",
                            "opcode": "EventSemaphore",
                            "outs": [],
                            "sync_info": {"on_update": [], "on_wait": [w]},
                        })
                    si["on_wait"] = ow[:cap]
                out.append(inst)
            blk["instructions"] = out
    return json.dumps(d).encode()


class _ProjEmitter:
    """Emits one layer's x-projection phase as a sequence of small units so
    the recurrence loop can interleave them between steps.

    Unit granularity: one (gt, dir) block = xpad prep (first dir only) +
    4 sub-tiles x 4 cb_o accumulation groups + the zs->DRAM store.
    Split into `units_per_dirblock` callables.
    """

    # (gt, d) tiles that stay live through the whole recurrence: the far-end
    # tile each direction consumes last
    HEAD_TAGS = {(7, "f"), (0, "b")}

    def __init__(self, nc, layer, units, xsrc, wx_sb, bias_sb, zs_tiles, pools, G):
        self.nc = nc
        self.layer = layer      # 0: xsrc is (xcm dram, xpad tiles); 1: (h0store, h0pad tiles)
        self.xsrc = xsrc
        self.wx_sb = wx_sb
        self.bias_sb = bias_sb
        self.zs_tiles = zs_tiles  # shared registry {(gt, d): SBUF tile}
        self.pools = pools      # dict: zp (PSUM), zs ring (SBUF), zs_head (SBUF)
        self.G = G
        self.units = list(units)          # explicit (gt, d, sub) order
        self.pos = 0
        self.block_buf = {}               # (gt, d) -> stage buffer index
        self.emitted_subs = {}            # (gt, d) -> set of emitted subs

    def _prep(self, gt, buf):
        """Stage gt's conv input unpadded: clipped taps never read a border."""
        nc = self.nc
        GT = 128
        ga = gt * GT
        stages = self.xsrc[1]
        xp = stages[buf]
        if self.layer == 0:
            xcm = self.xsrc[0]
            nc.sync.dma_start(out=xp[:], in_=xcm[:, :, ga : ga + GT, :])
        else:
            h0store = self.xsrc[0]
            nc.vector.tensor_add(
                xp[:],
                h0store[:, 0, ga : ga + GT, :],
                h0store[:, 1, ga : ga + GT, :],
            )
        return xp

    def emit_unit(self):
        if self.pos >= len(self.units):
            return False
        nc = self.nc
        gt, d, sub = self.units[self.pos]
        self.pos += 1
        GT = 128
        ga = gt * GT
        if (gt, d) not in self.block_buf:   # first touch of this dir-block
            buf = len(self.block_buf) % 2
            self.block_buf[(gt, d)] = buf
            self.emitted_subs[(gt, d)] = set()
            self._prep(gt, buf)
            if (gt, d) in self.HEAD_TAGS:
                self.zs_tiles[(gt, d)] = self.pools["zs_head"].tile(
                    [128, 4, GT, 9], BF16, name=f"zsh{gt}{d}", tag=f"zsh{gt}{d}"
                )
            else:
                self.zs_tiles[(gt, d)] = self.pools["zs"].tile(
                    [128, 4, GT, 9], BF16, name=f"zs{d}", tag=f"zs{d}"
                )
        buf = self.block_buf[(gt, d)]
        self.emitted_subs[(gt, d)].add(sub)
        xp = self.xsrc[1][buf]
        zs = self.zs_tiles[(gt, d)]
        g0 = sub * 32
        for cb_o in range(4):
            zp = self.pools["zp"].tile([128, 32, 3, 3], F32, name="azp", tag="azp")
            zpf = zp[:].rearrange("p g y x -> p (g y x)")
            k = 0
            n_k = 18 if self.layer == 0 else 9
            for dy, dx in TAPS:
                py, sy, ny = _clip(dy)
                px, sx, nx2 = _clip(dx)
                full = ny == 3 and nx2 == 3
                for cb_i in range(2 if self.layer == 0 else 1):
                    if self.layer == 0:
                        w_ap = self.wx_sb[d][
                            :, cb_i, dy * 3 + dx, cb_o * 128 : (cb_o + 1) * 128
                        ]
                        xv = xp[:, cb_i, g0 : g0 + 32, :].rearrange(
                            "p g (y x) -> p g y x", y=3, x=3
                        )
                    else:
                        w_ap = self.wx_sb[d][:, dy * 3 + dx, cb_o * 128 : (cb_o + 1) * 128]
                        xv = xp[:, g0 : g0 + 32, :].rearrange(
                            "p g (y x) -> p g y x", y=3, x=3
                        )
                    r_ap = xv if full else xv[:, :, sy : sy + ny, sx : sx + nx2]
                    o_ap = zpf if full else zp[:, :, py : py + ny, px : px + nx2]
                    nc.tensor.matmul(o_ap, w_ap, r_ap, start=(k == 0), stop=(k == n_k - 1))
                    k += 1
            # PSUM -> SBUF bf16 with bias fold; split ACT/DVE for balance
            dst = zs[:, cb_o, g0 : g0 + 32, :]
            srcv = zp[:].rearrange("p g y x -> p g (y x)")
            bias_ap = self.bias_sb[d][:, cb_o : cb_o + 1]
            if cb_o < 2:
                nc.scalar.activation(dst, srcv, IDENT, bias=bias_ap)
            else:
                nc.vector.tensor_scalar_add(dst, srcv, bias_ap)
        return True


def _recurrence(nc, tc, T, wh_sb, ident_bf, zs_tiles, hstore, pools, emitter, fill_rate, name,
                late_units=None):
    """One bidirectional ConvLSTM recurrence with interleaved filler units.
    zs_tiles: {(gt, d): SBUF tile [128, 4, 128, 9]} holding x-projections,
    filled by the emitters (head tiles before the loop, the rest JIT)."""
    stp, psp, gp = pools["st"], pools["ps"], pools["g"]
    cst = stp.tile([128, 2, BL * 9], BF16, name=f"{name}c", tag=f"{name}c")
    nc.gpsimd.memset(cst[:], 0.0)


    # PSUM gate-slot order (g, i, f, o): shortens the critical path --
    # tanh(g) is ready first, sigma(i,f) next, sigma(o) only needed for h
    CBS = (3, 0, 1, 2)

    def emit_taps(s, d):
        t = s if d == "f" else T - 1 - s
        tp = t - 1 if d == "f" else t + 1   # previous h timestep
        di = 0 if d == "f" else 1
        if emitter is not None:
            # build-time safety: the zs sub-tile we read must already be
            # emitted (program order IS the dependency order for the tracker)
            assert (t % 16) // 4 in emitter.emitted_subs[(t // 16, d)], (s, d, t)
        zxc = zs_tiles[(t // 16, d)]
        tof = t % 16
        zp = psp.tile([128, 512], F32, name=f"zp{d}", tag=f"{name}zp{d}")
        if s > 0:
            hv = hstore[:, di, tp * BL : (tp + 1) * BL, :].rearrange(
                "p b (y x) -> p b y x", y=3, x=3
            )
        for slot, cb in enumerate(CBS):
            gate = zp[:, slot * 72 : (slot + 1) * 72]
            nc.tensor.matmul(
                gate, ident_bf[:],
                zxc[:, cb, tof * BL : (tof + 1) * BL, :].rearrange("p b yx -> p (b yx)"),
                start=True, stop=(s == 0),
            )
            if s == 0:
                continue   # h(-1) == 0: the conv term vanishes
            g4 = gate.rearrange("p (b y x) -> p b y x", y=3, x=3)
            for kt, (dy, dx) in enumerate(TAPS):
                py, sy, ny = _clip(dy)
                px, sx, nx2 = _clip(dx)
                full = ny == 3 and nx2 == 3
                w_ap = wh_sb[d][:, dy * 3 + dx, cb * 128 : (cb + 1) * 128]
                r_ap = hv if full else hv[:, :, sy : sy + ny, sx : sx + nx2]
                o_ap = gate if full else g4[:, :, py : py + ny, px : px + nx2]
                nc.tensor.matmul(o_ap, w_ap, r_ap, start=False, stop=(kt == 8))
        return zp

    def emit_gates(s, d, zp):
        di = 0 if d == "f" else 1
        t = s if d == "f" else T - 1 - s
        # ACT order: tg, sigma(i,f), sigma(o), then tanh(c) after the c-update
        tg = gp.tile([128, BL * 9], BF16, name=f"tg{d}", tag=f"{name}tg{d}")
        sif = gp.tile([128, 2, BL * 9], BF16, name=f"sif{d}", tag=f"{name}sif{d}")
        so = gp.tile([128, BL * 9], BF16, name=f"so{d}", tag=f"{name}so{d}")
        nc.scalar.activation(tg[:], zp[:, 0:72], TANH)
        nc.scalar.activation(
            sif[:].rearrange("p g b -> p (g b)"), zp[:, 72:216], SIG
        )
        nc.scalar.activation(so[:], zp[:, 216:288], SIG)
        ig = gp.tile([128, BL * 9], BF16, name=f"ig{d}", tag=f"{name}ig{d}")
        cf = gp.tile([128, BL * 9], BF16, name=f"cf{d}", tag=f"{name}cf{d}")
        nc.vector.tensor_mul(ig[:], sif[:, 0], tg[:])
        nc.vector.tensor_mul(cf[:], sif[:, 1], cst[:, di])
        nc.vector.tensor_add(cst[:, di], ig[:], cf[:])
        tct = gp.tile([128, BL * 9], BF16, name=f"tc{d}", tag=f"{name}tc{d}")
        nc.scalar.activation(tct[:], cst[:, di], TANH)
        hsl = hstore[:, di, t * BL : (t + 1) * BL, :].rearrange("p b yx -> p (b yx)")
        nc.vector.tensor_mul(hsl, so[:], tct[:])

    # half-step software pipeline: emit taps(j), then gates(j-1) -- keeps the
    # two directions' chains phase-offset on the shared ACT/DVE engines
    prev = None
    for j in range(2 * T):
        s, d = j // 2, ("f", "b")[j % 2]
        if d == "f":
            if emitter is not None:
                # just-in-time fill: emit each projection quarter-tile two
                # steps before the recurrence first consumes it
                while (emitter.pos < len(emitter.units)
                       and _unit_deadline(emitter.units[emitter.pos]) <= s + 2):
                    emitter.emit_unit()
            while late_units and late_units[0][0] <= s:
                late_units.pop(0)[1]()
        zp = emit_taps(s, d)
        if prev is not None:
            emit_gates(*prev)
        prev = (s, d, zp)
    emit_gates(*prev)
    # drain any unfinished filler units
    if emitter is not None:
        while emitter.emit_unit():
            pass


def _unit_deadline(u):
    """Step at which the recurrence first consumes this (gt, d, sub) zs
    quarter-tile: fwd walks t up, bwd walks t down from the far end."""
    gt, d, sub = u
    return 16 * gt + 4 * sub if d == "f" else 16 * (7 - gt) + 4 * (3 - sub)


def _unit_order():
    units = [(gt, d, sub) for gt in range(8) for d in ("f", "b") for sub in range(4)]
    units.sort(key=lambda u: (_unit_deadline(u), u[1] != "f", u[0]))
    return units


def build_program(T=T_FULL):
    """Build the per-core Bass program. Returns nc."""
    _patch_tile_drain()
    G = T * BL
    GT = 128
    n_gt = G // GT

    nc = bass.Bass()

    # ---- I/O ----
    xcm = nc.dram_tensor("xcm", [128, 2, G, 9], BF16, kind="ExternalInput")
    wx0 = {}
    wh0 = {}
    wx1 = {}
    wh1 = {}
    bias_in = {}
    for d in ("f", "b"):
        wx0[d] = nc.dram_tensor(f"wx0{d}", [128, 2, 9, 512], BF16, kind="ExternalInput")
        wh0[d] = nc.dram_tensor(f"wh0{d}", [128, 9, 512], BF16, kind="ExternalInput")
        wx1[d] = nc.dram_tensor(f"wx1{d}", [128, 9, 512], BF16, kind="ExternalInput")
        wh1[d] = nc.dram_tensor(f"wh1{d}", [128, 9, 512], BF16, kind="ExternalInput")
        bias_in[f"0{d}"] = nc.dram_tensor(f"bias0{d}", [128, 4], F32, kind="ExternalInput")
        bias_in[f"1{d}"] = nc.dram_tensor(f"bias1{d}", [128, 4], F32, kind="ExternalInput")
    fcw = nc.dram_tensor("fcw", [128, 9, NCLS], BF16, kind="ExternalInput")
    fcb = nc.dram_tensor("fcb", [NCLS, 1], F32, kind="ExternalInput")
    out = nc.dram_tensor("out", [NCLS, G], F32, kind="ExternalOutput")

    with TileContext(nc) as tc:
        with tc.tile_pool(name="persist", bufs=1) as pp:
            h0store = pp.tile([128, 2, G, 9], BF16, tag="h0store")

            # ===================== layer 0 (A + B) =====================
            with (
                tc.tile_pool(name="l0w", bufs=1) as wp0,
                tc.tile_pool(name="a_zs", bufs=3) as azsp,
                tc.tile_pool(name="a_zsh", bufs=1) as azshp,
                tc.tile_pool(name="a_zp", bufs=3, space="PSUM") as azpp,
                tc.tile_pool(name="b_st", bufs=1) as bstp,
                tc.tile_pool(name="b_ps", bufs=2, space="PSUM") as bpsp,
                tc.tile_pool(name="b_g", bufs=2) as bgp,
            ):
                wx0_sb = {}
                wh0_sb = {}
                wx0_sb["f"] = wp0.tile([128, 2, 9, 512], BF16, name="wx0f", tag="wx0f")
                nc.sync.dma_start(out=wx0_sb["f"][:], in_=wx0["f"][:])
                bias_sb = {}
                for d in ("f", "b"):
                    for l in ("0", "1"):
                        bias_sb[l + d] = pp.tile([128, 4], F32, name=f"bias{l}{d}", tag=f"bias{l}{d}")
                        nc.sync.dma_start(out=bias_sb[l + d][:], in_=bias_in[l + d][:])
                fcb_sb = pp.tile([NCLS, 1], F32, tag="fcb")
                nc.sync.dma_start(out=fcb_sb[:], in_=fcb[:])
                ident_bf = pp.tile([128, 128], BF16, tag="identbf")
                make_identity(nc, ident_bf[:])
                xstage = [
                    wp0.tile([128, 2, GT, 9], BF16, name=f"xst{par}", tag=f"xst{par}")
                    for par in range(2)
                ]

                apools = {"zp": azpp, "zs": azsp, "zs_head": azshp}
                zst0 = {}
                em0 = _ProjEmitter(
                    nc, 0, _unit_order(), (xcm, xstage), wx0_sb,
                    {d: bias_sb["0" + d] for d in ("f", "b")}, zst0, apools, G,
                )
                em0.emit_unit()          # (0, "f", 0): needs only wx0f
                wx0_sb["b"] = wp0.tile([128, 2, 9, 512], BF16, name="wx0b", tag="wx0b")
                nc.sync.dma_start(out=wx0_sb["b"][:], in_=wx0["b"][:])
                for d in ("f", "b"):
                    wh0_sb[d] = wp0.tile([128, 9, 512], BF16, name=f"wh0{d}", tag=f"wh0{d}")
                    nc.sync.dma_start(out=wh0_sb[d][:], in_=wh0[d][:])
                bpools = {"st": bstp, "ps": bpsp, "g": bgp}
                _recurrence(
                    nc, tc, T, wh0_sb, ident_bf, zst0, h0store, bpools,
                    em0, 2.0, "l0",
                )

            # ===================== layer 1 (C + D + E) =====================
            with tc.tile_pool(name="l1w", bufs=1) as wp1:
                wx1_sb = {}
                wh1_sb = {}
                for d in ("f", "b"):
                    wx1_sb[d] = wp1.tile([128, 9, 512], BF16, name=f"wx1{d}", tag=f"wx1{d}")
                    nc.sync.dma_start(out=wx1_sb[d][:], in_=wx1[d][:])
                    wh1_sb[d] = wp1.tile([128, 9, 512], BF16, name=f"wh1{d}", tag=f"wh1{d}")
                    nc.sync.dma_start(out=wh1_sb[d][:], in_=wh1[d][:])
                fcw_sb = wp1.tile([128, 9, NCLS], BF16, tag="fcw")
                nc.sync.dma_start(out=fcw_sb[:], in_=fcw[:])
                h1store = wp1.tile([128, 2, G, 9], BF16, tag="h1store")
                h0stage = [
                    wp1.tile([128, GT, 9], BF16, name=f"h0st{par}", tag=f"h0st{par}")
                    for par in range(2)
                ]

                with (
                    tc.tile_pool(name="c_zs", bufs=3) as czsp,
                    tc.tile_pool(name="c_zsh", bufs=1) as czshp,
                    tc.tile_pool(name="c_zp", bufs=3, space="PSUM") as czpp,
                    tc.tile_pool(name="d_st", bufs=1) as dstp,
                    tc.tile_pool(name="d_ps", bufs=2, space="PSUM") as dpsp,
                    tc.tile_pool(name="d_g", bufs=2) as dgp,
                ):
                    cpools = {"zp": czpp, "zs": czsp, "zs_head": czshp}
                    zst1 = {}
                    em1 = _ProjEmitter(
                        nc, 1, _unit_order(), (h0store, h0stage), wx1_sb,
                        {d: bias_sb["1" + d] for d in ("f", "b")}, zst1, cpools, G,
                    )
                    em1b = em1
                    dpools = {"st": dstp, "ps": dpsp, "g": dgp}
                    # ========== Phase E pools open across D so mid chunks can
                    # fill D's tail idle (readiness-ordered) ==========
                    with (
                        tc.tile_pool(name="e_h", bufs=2) as ehp,
                        tc.tile_pool(name="e_ps", bufs=1, space="PSUM") as epp,
                        tc.tile_pool(name="e_o", bufs=2) as eop,
                    ):
                        EC = 128

                        def e_chunk(gc):
                            ga = gc * EC
                            h1s = ehp.tile([128, EC, 9], BF16, name="eh1s", tag="eh1s")
                            nc.vector.tensor_add(
                                h1s[:],
                                h1store[:, 0, ga : ga + EC, :],
                                h1store[:, 1, ga : ga + EC, :],
                            )
                            ps = epp.tile([NCLS, EC], F32, name="eps", tag="eps")
                            for yx in range(9):
                                nc.tensor.matmul(
                                    ps[:],
                                    fcw_sb[:, yx, :],
                                    h1s[:, :, yx],
                                    start=(yx == 0),
                                    stop=(yx == 8),
                                )
                            ot = eop.tile([NCLS, EC], F32, name="eo", tag="eo")
                            nc.vector.tensor_scalar_add(ot[:], ps[:], fcb_sb[:, 0:1])
                            nc.sync.dma_start(out=out[:, ga : ga + EC], in_=ot[:])

                        late = [(81, lambda: e_chunk(3)), (82, lambda: e_chunk(4)),
                                (97, lambda: e_chunk(2)), (98, lambda: e_chunk(5)),
                                (113, lambda: e_chunk(1)), (114, lambda: e_chunk(6))]
                        _recurrence(
                            nc, tc, T, wh1_sb, ident_bf, zst1, h1store, dpools,
                            em1b, 2.0, "l1", late_units=late,
                        )
                        e_chunk(0)
                        e_chunk(7)

    _orig_to_json = nc.to_json_bytes
    nc.to_json_bytes = lambda: _fix_multi_waits(_orig_to_json())
    return nc


# ---------------- host side ----------------

def _prep_weights(w, b, cin):
    """w: (512, cin+128, 3, 3) -> (wx, wh) bf16 host arrays + bias (128,4) f32."""
    bf = ml_dtypes.bfloat16
    wx = w[:, :cin].reshape(512, cin, 9)            # (co, ci, tap)
    wx = wx.transpose(1, 2, 0)                      # (ci, tap, co)
    if cin == 256:
        wx = wx.reshape(2, 128, 9, 512).transpose(1, 0, 2, 3)  # (128, 2, 9, 512)
    wx = np.ascontiguousarray(wx).astype(bf)
    wh = w[:, cin:].reshape(512, 128, 9).transpose(1, 2, 0)    # (128, 9, 512)
    wh = np.ascontiguousarray(wh).astype(bf)
    bias = np.ascontiguousarray(b.reshape(4, 128).T).astype(np.float32)
    return wx, wh, bias


def make_inputs_core(core, x, w_f0, b_f0, w_b0, b_b0, w_f1, b_f1, w_b1, b_b1,
                     fc_w, fc_b):
    bf = ml_dtypes.bfloat16
    xs = np.ascontiguousarray(x[core * BL : (core + 1) * BL])  # (BL, T, 256, 3, 3)
    T = xs.shape[1]
    xcm = xs.reshape(BL, T, 2, 128, 9).transpose(3, 2, 1, 0, 4)  # (128, 2, T, BL, 9)
    xcm = np.ascontiguousarray(xcm.reshape(128, 2, T * BL, 9)).astype(bf)
    m = {"xcm": xcm}
    for d, w, b in (("f", w_f0, b_f0), ("b", w_b0, b_b0)):
        wx, wh, bias = _prep_weights(np.asarray(w), np.asarray(b), 256)
        m[f"wx0{d}"], m[f"wh0{d}"], m[f"bias0{d}"] = wx, wh, bias
    for d, w, b in (("f", w_f1, b_f1), ("b", w_b1, b_b1)):
        wx, wh, bias = _prep_weights(np.asarray(w), np.asarray(b), 128)
        m[f"wx1{d}"], m[f"wh1{d}"], m[f"bias1{d}"] = wx, wh, bias
    fcw = np.asarray(fc_w).reshape(NCLS, 128, 9).transpose(1, 2, 0)  # (128, 9, 7)
    m["fcw"] = np.ascontiguousarray(fcw).astype(bf)
    m["fcb"] = np.ascontiguousarray(np.asarray(fc_b).reshape(NCLS, 1)).astype(np.float32)
    return m


_nc_cache = {}


def kernel(**inputs):
    from concourse.bass_utils import run_bass_kernel_spmd

    if "nc" not in _nc_cache:
        _nc_cache["nc"] = build_program(T_FULL)
    nc = _nc_cache["nc"]
    x = np.asarray(inputs["x"], dtype=np.float32)
    in_maps = [make_inputs_core(c, x, inputs["w_f0"], inputs["b_f0"],
                                inputs["w_b0"], inputs["b_b0"],
                                inputs["w_f1"], inputs["b_f1"],
                                inputs["w_b1"], inputs["b_b1"],
                                inputs["fc_w"], inputs["fc_b"])
               for c in range(NCORES)]
    res = run_bass_kernel_spmd(nc, in_maps, core_ids=list(range(NCORES)))
    outs = []
    for c in range(NCORES):
        o = res.results[c]["out"]  # (7, G) with g = t*BL + b
        o = o.reshape(NCLS, T_FULL, BL).transpose(2, 1, 0)  # (BL, T, 7)
        outs.append(o)
    return np.ascontiguousarray(np.concatenate(outs, axis=0), dtype=np.float32)
